# revision 12
# baseline (speedup 1.0000x reference)
"""Multi-head attention (B=32, T=512, E=768, H=12) on 8 trn2 NeuronCores.

Sharding: data-parallel over batch (4 batches per core). Weights replicated.
All matmuls run as float32r (full-rate fp32 streaming on the PE).

Per-core layout strategy:
  - host feeds xT [4, E, T] (pre-transposed), W.T [E, E] for Q/K/O, and an
    augmented V weight [E, 780] with per-head column blocks of 65 (64 data
    columns + 1 zero column whose bias entry is 1.0 -> a ready-made "ones"
    column for softmax row sums).
  - Q/K projections produce QT/KT [E, T] with head-dim on partitions; bias
    applied by DVE tensor_scalar (per-partition) while rounding to f32r.
  - V lands in natural [t, head-blocks] layout; the attention mask (0/1 per
    key) is multiplied into V rows (including the ones column), which is
    exactly equivalent to -inf score masking after renormalization.
  - scores are computed transposed: scoresT[k, q] = sum_d K[k,d] Q[q,d],
    with even/odd head pairs issued back-to-back as PE row-tile pairs
    (K=64 each, base partitions 0/64) so they execute concurrently.
  - softmax: exp on ScalarE over two k-tiles per instruction; 1/sqrt(d)
    folded into the activation scale.
  - ctxT[d, q] + sums row = matmul(lhsT=[V_h | 1], rhs=probsT) accumulated
    over k tiles. Normalization per head pair: two DVE reciprocals into one
    [2, 512] tile, one rank-2 PE broadcast through a host-fed selector
    matrix, one DVE multiply, one DVE f32r copy into mergedT.
  - output projection consumes mergedT tiles as lhsT; bias added by DVE
    from PE-broadcast bias tiles; result DMA'd out in natural layout.
"""

import numpy as np

import concourse.bass as bass
import concourse.mybir as mybir
import concourse.tile as tile
from concourse import bacc
from concourse.bass_utils import run_bass_kernel_spmd

F32 = mybir.dt.float32
F32R = mybir.dt.float32r
AF = mybir.ActivationFunctionType
ALU = mybir.AluOpType

N_CORES = 8
B, T, E = 32, 512, 768
H, D = 12, 64
BPC = B // N_CORES          # batches per core
TT = T // 128               # token tiles per batch (4)
ET = E // 128               # embed tiles (6)
VW = H * 66                 # augmented V width (792), 66 = even (fp32r ISA)
VC = ((0, 462), (462, 330))  # V output chunks (7 + 5 head blocks)


def build_nc():
    nc = bacc.Bacc("TRN2", target_bir_lowering=False, num_devices=N_CORES)

    xt = nc.dram_tensor("xt", [BPC, 128, ET, T], F32, kind="ExternalInput")
    wqt = nc.dram_tensor("wqt", [128, ET, E], F32, kind="ExternalInput")
    wkt = nc.dram_tensor("wkt", [128, ET, E], F32, kind="ExternalInput")
    wvta = nc.dram_tensor("wvta", [128, ET, VW], F32, kind="ExternalInput")
    wot = nc.dram_tensor("wot", [128, ET, E], F32, kind="ExternalInput")
    bq2 = nc.dram_tensor("bq2", [128, ET], F32, kind="ExternalInput")
    bk2 = nc.dram_tensor("bk2", [128, ET], F32, kind="ExternalInput")
    bvp = nc.dram_tensor("bvp", [VW], F32, kind="ExternalInput")
    bo = nc.dram_tensor("bo", [E], F32, kind="ExternalInput")
    maskf = nc.dram_tensor("maskf", [BPC, 128, TT], F32, kind="ExternalInput")
    sel2 = nc.dram_tensor("sel2", [65, 128], F32, kind="ExternalInput")
    ones = nc.dram_tensor("ones", [128], F32, kind="ExternalInput")
    out = nc.dram_tensor("out", [BPC, T, E], F32, kind="ExternalOutput")

    with tile.TileContext(nc) as tc, nc.allow_low_precision(
        "float32r tiles feed the PE; rounding to f32r is intentional"
    ):
        with (
            tc.tile_pool(name="consts", bufs=1) as consts,
            tc.tile_pool(name="work", bufs=1) as work,
            tc.tile_pool(name="pp", bufs=2, space="PSUM") as pp,
            tc.tile_pool(name="sc", bufs=2, space="PSUM") as sc,
            tc.tile_pool(name="cx", bufs=2, space="PSUM") as cx,
        ):
            # ---- input DMAs: xt batches 0/1 on the Sync queue first, then
            # weights on the Scalar HWDGE queue so they stream in parallel.
            def load_xt(b):
                ts = []
                for et in range(ET):
                    t = work.tile([128, T], F32R, name=f"xt{b}_{et}",
                                  tag=f"xt{et}", bufs=2)
                    nc.sync.dma_start(t[:], xt[b, :, et, :].bitcast(F32R))
                    ts.append(t)
                return ts

            xt_tiles = [load_xt(b) for b in range(min(2, BPC))]

            def load_weight(wname, w_dr, width):
                tiles = []
                for et in range(ET):
                    t = consts.tile([128, width], F32R, name=f"{wname}{et}")
                    nc.scalar.dma_start(t[:], w_dr[:, et, :].bitcast(F32R))
                    tiles.append(t)
                return tiles

            wq_sb = load_weight("wq_sb", wqt, E)
            # stagger the remaining weight streams so they don't steal DMA
            # bandwidth from the critical wq/xt0 tiles at kernel start
            with tc.tile_wait_until(0.010):
                wk_sb = load_weight("wk_sb", wkt, E)
            with tc.tile_wait_until(0.016):
                wv_sb = load_weight("wv_sb", wvta, VW)
            with tc.tile_wait_until(0.022):
                wo_sb = load_weight("wo_sb", wot, E)

            bq_sb = consts.tile([128, ET], F32, name="bq_sb")
            bk_sb = consts.tile([128, ET], F32, name="bk_sb")
            nc.scalar.dma_start(bq_sb[:], bq2[:, :])
            nc.scalar.dma_start(bk_sb[:], bk2[:, :])

            sel2_sb = consts.tile([65, 128], F32R, name="sel2_sb")
            nc.scalar.dma_start(sel2_sb[:], sel2[:, :].bitcast(F32R))
            ones_sb = consts.tile([1, 128], F32R, name="ones_sb")
            nc.scalar.dma_start(
                ones_sb[:], ones.rearrange("(p o) -> p o", p=1).bitcast(F32R)
            )
            bvp_row = consts.tile([1, VW], F32R, name="bvp_row")
            nc.scalar.dma_start(
                bvp_row[:], bvp.rearrange("(p o) -> p o", p=1).bitcast(F32R)
            )
            bo_row = consts.tile([1, E], F32R, name="bo_row")
            nc.scalar.dma_start(
                bo_row[:], bo.rearrange("(p o) -> p o", p=1).bitcast(F32R)
            )

            # broadcast bias rows across partitions via rank-1 matmuls
            bvp_bc = consts.tile([128, VW], F32, name="bvp_bc")
            for cstart, clen in VC:
                ps = pp.tile([128, 512], F32, name="bc_ps", tag="pp")
                nc.tensor.matmul(ps[:, :clen], ones_sb[:],
                                 bvp_row[:, cstart:cstart + clen],
                                 start=True, stop=True)
                nc.scalar.activation(out=bvp_bc[:, cstart:cstart + clen],
                                     in_=ps[:, :clen], func=AF.Copy)
            bo_bc = consts.tile([128, E], F32, name="bo_bc")
            for cstart, clen in ((0, 512), (512, 256)):
                ps = pp.tile([128, 512], F32, name="bc_ps2", tag="pp")
                nc.tensor.matmul(ps[:, :clen], ones_sb[:],
                                 bo_row[:, cstart:cstart + clen],
                                 start=True, stop=True)
                nc.scalar.activation(out=bo_bc[:, cstart:cstart + clen],
                                     in_=ps[:, :clen], func=AF.Copy)

            # ---- per-batch ------------------------------------------------
            for b in range(BPC):
                xt_sb = xt_tiles[b]
                if b + 2 < BPC:
                    xt_tiles.append(load_xt(b + 2))

                mk = work.tile([128, TT], F32, name="mk", bufs=2)
                nc.sync.dma_start(mk[:], maskf[b, :, :])

                # --- Q/K projections -> QT/KT [E, T] ----------------------
                qt_sb = work.tile([128, ET, T], F32R, name="qt_sb")
                kt_sb = work.tile([128, ET, T], F32R, name="kt_sb")
                for dst, w_sb, b_sb in ((qt_sb, wq_sb, bq_sb),
                                        (kt_sb, wk_sb, bk_sb)):
                    for ot in range(ET):
                        ps = pp.tile([128, 512], F32, name="proj_ps", tag="pp")
                        for et in range(ET):
                            nc.tensor.matmul(
                                ps[:],
                                w_sb[et][:, ot * 128:(ot + 1) * 128],
                                xt_sb[et][:],
                                start=(et == 0), stop=(et == ET - 1),
                            )
                        nc.vector.tensor_scalar_add(
                            dst[:, ot, :], ps[:], b_sb[:, ot:ot + 1]
                        )

                # --- V projection -> [t, head-blocks] + masked rows -------
                v_sb = work.tile([128, TT, VW], F32R, name="v_sb")
                for tt in range(TT):
                    for cstart, clen in VC:
                        ps = pp.tile([128, 512], F32, name="vproj_ps", tag="pp")
                        for et in range(ET):
                            nc.tensor.matmul(
                                ps[:, :clen],
                                xt_sb[et][:, tt * 128:(tt + 1) * 128],
                                wv_sb[et][:, cstart:cstart + clen],
                                start=(et == 0), stop=(et == ET - 1),
                            )
                        nc.vector.tensor_add(
                            ps[:, :clen], ps[:, :clen],
                            bvp_bc[:, cstart:cstart + clen],
                        )
                        nc.vector.tensor_scalar_mul(
                            v_sb[:, tt, cstart:cstart + clen],
                            ps[:, :clen], mk[:, tt:tt + 1],
                        )

                # --- attention, even/odd head pairs -----------------------
                merged_sb = work.tile([128, ET, T], F32R, name="merged_sb")
                for j in range(H // 2):
                    probs = []
                    for half in range(2):
                        spsE = sc.tile([128, 2, 512], F32, name="spsE", tag="sc")
                        spsO = sc.tile([128, 2, 512], F32, name="spsO", tag="sc")
                        for kk in range(2):
                            kt = half * 2 + kk
                            ksl = slice(kt * 128, (kt + 1) * 128)
                            nc.tensor.matmul(
                                spsE[:, kk, :], kt_sb[0:64, j, ksl],
                                qt_sb[0:64, j, :], start=True, stop=True,
                            )
                            nc.tensor.matmul(
                                spsO[:, kk, :], kt_sb[64:128, j, ksl],
                                qt_sb[64:128, j, :], start=True, stop=True,
                            )
                        pE = work.tile([128, 2, 512], F32R, name="probsE",
                                       tag="probsE", bufs=2)
                        pO = work.tile([128, 2, 512], F32R, name="probsO",
                                       tag="probsO", bufs=2)
                        nc.scalar.activation(out=pE[:], in_=spsE[:],
                                             func=AF.Exp, scale=0.125)
                        nc.scalar.activation(out=pO[:], in_=spsO[:],
                                             func=AF.Exp, scale=0.125)
                        probs.append((pE, pO))
                    cpsE = cx.tile([66, 512], F32, name="cpsE", tag="cx")
                    cpsO = cx.tile([66, 512], F32, name="cpsO", tag="cx")
                    for kt in range(TT):
                        pE, pO = probs[kt // 2]
                        kk = kt % 2
                        nc.tensor.matmul(
                            cpsE[:], v_sb[:, kt, 2 * j * 66:(2 * j + 1) * 66],
                            pE[:, kk, :], start=(kt == 0), stop=(kt == TT - 1),
                        )
                        nc.tensor.matmul(
                            cpsO[:], v_sb[:, kt, (2 * j + 1) * 66:(2 * j + 2) * 66],
                            pO[:, kk, :], start=(kt == 0), stop=(kt == TT - 1),
                        )
                    rr2 = work.tile([65, 512], F32R, name="rr2", tag="rr2", bufs=2)
                    nc.vector.tensor_copy(rr2[0:1, :], cpsE[64:65, :])
                    nc.vector.tensor_copy(rr2[64:65, :], cpsO[64:65, :])
                    um = work.tile([128, 512], F32, name="um", tag="um", bufs=2)
                    nc.scalar.activation(out=um[0:64, :], in_=cpsE[0:64, :],
                                         func=AF.Copy)
                    nc.scalar.activation(out=um[64:128, :], in_=cpsO[0:64, :],
                                         func=AF.Copy)
                    bps = pp.tile([128, 512], F32, name="bps", tag="pp")
                    nc.tensor.matmul(bps[:], sel2_sb[0:1, :], rr2[0:1, :],
                                     start=True, stop=False)
                    nc.tensor.matmul(bps[:], sel2_sb[64:65, :], rr2[64:65, :],
                                     start=False, stop=True)
                    rb = work.tile([128, 512], F32, name="rb", tag="rb", bufs=2)
                    nc.vector.reciprocal_approx_fast(out=rb[:], in_=bps[:])
                    nc.vector.tensor_mul(um[:], um[:], rb[:])
                    nc.vector.tensor_copy(merged_sb[:, j, :], um[:])

                # --- output projection ------------------------------------
                for tt in range(TT):
                    o_sb = work.tile([128, E], F32, name="o_sb",
                                     tag="o_sb", bufs=2)
                    for cstart, clen in ((0, 512), (512, 256)):
                        ps = pp.tile([128, 512], F32, name="oproj_ps", tag="pp")
                        for mt in range(ET):
                            nc.tensor.matmul(
                                ps[:, :clen],
                                merged_sb[:, mt, tt * 128:(tt + 1) * 128],
                                wo_sb[mt][:, cstart:cstart + clen],
                                start=(mt == 0), stop=(mt == ET - 1),
                            )
                        nc.vector.tensor_add(
                            o_sb[:, cstart:cstart + clen],
                            ps[:, :clen],
                            bo_bc[:, cstart:cstart + clen],
                        )
                    nc.sync.dma_start(
                        out[b, tt * 128:(tt + 1) * 128, :], o_sb[:]
                    )

    nc.finalize()
    return nc


_NC = None


def _get_nc():
    global _NC
    if _NC is None:
        _NC = build_nc()
    return _NC


def make_in_maps(x, attention_mask, wq, bq, wk, bk, wv, bv, wo, bo):
    x = np.asarray(x, dtype=np.float32)
    attention_mask = np.asarray(attention_mask)

    def wshuf(w):
        # [o, e] -> [p, et, o] with e = et*128 + p (partition-contiguous DMA)
        return np.ascontiguousarray(
            np.asarray(w, dtype=np.float32).reshape(E, ET, 128).transpose(2, 1, 0)
        )

    wqt = wshuf(wq)
    wkt = wshuf(wk)
    wot = wshuf(wo)
    wvt = np.asarray(wv, dtype=np.float32).T          # [E(in), E(out)]
    bq = np.asarray(bq, dtype=np.float32)
    bk = np.asarray(bk, dtype=np.float32)
    bv = np.asarray(bv, dtype=np.float32)
    bo = np.ascontiguousarray(np.asarray(bo, dtype=np.float32))

    # augmented V weight: per-head 66-column blocks; column 64 is zero in
    # the weight and 1.0 in the bias -> yields the softmax-sum column.
    wvta_flat = np.zeros((E, VW), dtype=np.float32)
    bvp = np.zeros(VW, dtype=np.float32)
    for h in range(H):
        wvta_flat[:, h * 66:h * 66 + 64] = wvt[:, h * 64:(h + 1) * 64]
        bvp[h * 66:h * 66 + 64] = bv[h * 64:(h + 1) * 64]
        bvp[h * 66 + 64] = 1.0
    wvta = np.ascontiguousarray(
        wvta_flat.reshape(ET, 128, VW).transpose(1, 0, 2)
    )

    bq2 = np.ascontiguousarray(bq.reshape(ET, 128).T)
    bk2 = np.ascontiguousarray(bk.reshape(ET, 128).T)
    sel2 = np.zeros((65, 128), dtype=np.float32)
    sel2[0, 0:64] = 1.0
    sel2[64, 64:128] = 1.0
    onesv = np.ones(128, dtype=np.float32)
    maskf_full = np.ascontiguousarray(
        np.asarray(attention_mask, dtype=np.float32)
        .reshape(B, TT, 128).transpose(0, 2, 1)
    )  # [B, 128, TT]

    in_maps = []
    for c in range(N_CORES):
        sl = slice(c * BPC, (c + 1) * BPC)
        in_maps.append({
            "xt": np.ascontiguousarray(
                x[sl].reshape(BPC, T, ET, 128).transpose(0, 3, 2, 1)
            ),
            "maskf": np.ascontiguousarray(maskf_full[sl]),
            "wqt": wqt, "wkt": wkt, "wvta": wvta, "wot": wot,
            "bq2": bq2, "bk2": bk2, "bvp": bvp, "bo": bo,
            "sel2": sel2, "ones": onesv,
        })
    return in_maps


def kernel(**inputs):
    in_maps = make_in_maps(**inputs)
    res = run_bass_kernel_spmd(_get_nc(), in_maps, core_ids=list(range(N_CORES)))
    return np.concatenate([res.results[c]["out"] for c in range(N_CORES)], axis=0)
